# revision 5
# baseline (speedup 1.0000x reference)
"""Trainium2 Bass kernel for nn_AttentionBlock (B=8, C=256, H=W=128).

Math (per batch element, data-parallel over 8 cores):
  xd = avgpool2x2(x)                      # [C, 64, 64] -> n=4096
  q = Wq xd + bq  (d=16);  k = Wk xd      # bk cancels in softmax (per-row const)
  S[n,m] = q_n . k_m ; attn = softmax_m(S) (no max-subtraction needed; |S| <~ 7)
  out = v @ attn^T where v = Wv xd + bv
  y = gamma*out upsampled bilinearly (half-pixel) + x

Device layout choices:
  - S is computed transposed (S^T[m,n]) so exp(S^T) tiles feed the output
    bmm directly as the moving operand with lhsT = v^T tiles.
  - softmax denominator via ones-vector matmuls accumulated in PSUM.
  - bias bv and gamma folded: Wv,bv pre-scaled by gamma on host; bv added
    post-normalization as a per-partition scalar; the 1/4 avgpool scale is
    folded into Wq/Wk/Wv host-side.
  - bilinear 2x upsample is separable: per axis out = 0.75*a + 0.25*a_shift,
    done with scalar_tensor_tensor ops; residual add on gpsimd.
"""

import sys

sys.path.insert(0, "/opt/trn_rl_repo")

import numpy as np
import ml_dtypes

import concourse.bass as bass
import concourse.tile as tile
from concourse import bacc, mybir
from concourse.bass_utils import run_bass_kernel_spmd

AF = mybir.ActivationFunctionType
ALU = mybir.AluOpType
BF16 = mybir.dt.bfloat16
F32 = mybir.dt.float32


def build_program(C=256, H=128, W=128, D=16, n_cores=8):
    assert C == 256 and D == 16
    h, w = H // 2, W // 2
    n = h * w
    CT = C // 128          # c tiles (2)
    MT = n // 128          # m tiles
    NBLK = min(512, n)     # n block size
    NB = n // NBLK         # n blocks
    GS = min(1024 // NBLK, MT)   # m-tiles per s-psum group (2 banks)
    SH = H // 16           # pooling strips (16 input rows each)
    SH2 = h // 8           # upsample strips (8 src rows -> 16 out rows)

    nc = bacc.Bacc("TRN2", target_bir_lowering=False, debug=False,
                   num_devices=n_cores)

    x = nc.dram_tensor("x", [C, H, W], F32, kind="ExternalInput").ap()
    wqt = nc.dram_tensor("wqt", [C, D], BF16, kind="ExternalInput").ap()
    wkt = nc.dram_tensor("wkt", [C, D], BF16, kind="ExternalInput").ap()
    wvt = nc.dram_tensor("wvt", [C, C], BF16, kind="ExternalInput").ap()
    bqv = nc.dram_tensor("bqv", [D, 1], F32, kind="ExternalInput").ap()
    gbv = nc.dram_tensor("gbv", [1, C], F32, kind="ExternalInput").ap()
    out = nc.dram_tensor("out", [C, H, W], F32, kind="ExternalOutput").ap()

    with tile.TileContext(nc) as tc:
        emit(tc, nc, x, wqt, wkt, wvt, bqv, gbv, out,
             C=C, H=H, W=W, D=D, h=h, w=w, n=n, CT=CT, MT=MT,
             NBLK=NBLK, NB=NB, GS=GS, SH=SH, SH2=SH2)

    nc.compile()
    return nc


def emit(tc, nc, x, wqt, wkt, wvt, bqv, gbv, out, *,
         C, H, W, D, h, w, n, CT, MT, NBLK, NB, GS, SH, SH2):
    MM = nc.tensor.matmul

    with tc.tile_pool(name="persist", bufs=1) as persist:
        # ---- persistent tiles ----
        xd = persist.tile([128, CT, n], BF16)          # pooled input, c-major
        q_sb = persist.tile([16, n], BF16)
        k_sb = persist.tile([16, n], BF16)
        vt_sb = persist.tile([128, MT, C], BF16)       # v^T tiles [m, c]
        wq_sb = persist.tile([128, CT, D], BF16)
        wk_sb = persist.tile([128, CT, D], BF16)
        wv_sb = persist.tile([128, CT, C], BF16)
        bq_sb = persist.tile([16, 1], F32)
        gbv_sb = persist.tile([128, CT], F32)          # gamma*bv per partition
        ones_sb = persist.tile([128, 1], BF16)
        outn = persist.tile([128, CT, n], F32)         # normalized attn output

        nc.sync.dma_start(out=wq_sb, in_=wqt.rearrange("(t p) d -> p t d", p=128))
        nc.sync.dma_start(out=wk_sb, in_=wkt.rearrange("(t p) d -> p t d", p=128))
        nc.sync.dma_start(out=wv_sb, in_=wvt.rearrange("(t p) c -> p t c", p=128))
        nc.sync.dma_start(out=bq_sb, in_=bqv)
        nc.sync.dma_start(out=gbv_sb, in_=gbv.rearrange("o (t p) -> p (t o)", p=128))
        nc.vector.memset(ones_sb, 1.0)

        # ---- phase A: load x, 2x2 average pool (scale folded into weights) ----
        with tc.tile_pool(name="xload", bufs=3) as xload:
            for ct in range(CT):
                for hs in range(SH):
                    xt = xload.tile([128, 16, W], F32, tag="xt")
                    nc.sync.dma_start(
                        out=xt,
                        in_=x[ct * 128:(ct + 1) * 128, hs * 16:(hs + 1) * 16, :])
                    t1 = xload.tile([128, 16, w], F32, tag="t1")
                    nc.gpsimd.tensor_add(t1, xt[:, :, 0::2], xt[:, :, 1::2])
                    xdv = xd[:, ct, hs * 8 * w:(hs + 1) * 8 * w].rearrange(
                        "p (r q) -> p r q", q=w)
                    nc.gpsimd.tensor_add(xdv, t1[:, 0::2, :], t1[:, 1::2, :])

        # ---- phase B: projections q, k, v^T ----
        with tc.tile_pool(name="bpsum", bufs=2, space="PSUM") as bpsum:
            for nb in range(NB):
                sl = slice(nb * NBLK, (nb + 1) * NBLK)
                qp = bpsum.tile([16, NBLK], F32, tag="qk")
                MM(qp, wq_sb[:, 0, :], xd[:, 0, sl], start=True, stop=(CT == 1))
                if CT > 1:
                    MM(qp, wq_sb[:, 1, :], xd[:, 1, sl], start=False, stop=True)
                nc.scalar.activation(q_sb[:, sl], qp, AF.Identity, bias=bq_sb)
                kp = bpsum.tile([16, NBLK], F32, tag="qk")
                MM(kp, wk_sb[:, 0, :], xd[:, 0, sl], start=True, stop=(CT == 1))
                if CT > 1:
                    MM(kp, wk_sb[:, 1, :], xd[:, 1, sl], start=False, stop=True)
                nc.scalar.copy(k_sb[:, sl], kp)
            for mt in range(MT):
                msl = slice(mt * 128, (mt + 1) * 128)
                vp = bpsum.tile([128, C], F32, tag="v")
                MM(vp, xd[:, 0, msl], wv_sb[:, 0, :], start=True, stop=(CT == 1))
                if CT > 1:
                    MM(vp, xd[:, 1, msl], wv_sb[:, 1, :], start=False, stop=True)
                nc.scalar.copy(vt_sb[:, mt, :], vp)

        # ---- phase C: attention ----
        NG = (MT + GS - 1) // GS
        with tc.tile_pool(name="spsum", bufs=2, space="PSUM") as spsum, \
             tc.tile_pool(name="opsum", bufs=1, space="PSUM") as opsum, \
             tc.tile_pool(name="etp", bufs=2) as etp, \
             tc.tile_pool(name="nrm", bufs=2) as nrm:
            for nb in range(NB):
                nsl = slice(nb * NBLK, (nb + 1) * NBLK)
                o_ps = [opsum.tile([128, NBLK], F32, tag=f"o{ci}",
                                   name=f"o_ps{ci}_{nb}")
                        for ci in range(CT)]
                rs_ps = opsum.tile([1, NBLK], F32, tag="rs")

                def out_mms(mg, et):
                    for j in range(GS):
                        mt = mg * GS + j
                        if mt >= MT:
                            break
                        first, last = (mt == 0), (mt == MT - 1)
                        for ci in range(CT):
                            MM(o_ps[ci], vt_sb[:, mt, ci * 128:(ci + 1) * 128],
                               et[:, j, :], start=first, stop=last)
                        MM(rs_ps, ones_sb, et[:, j, :], start=first, stop=last)

                prev = None
                for mg in range(NG):
                    gs = min(GS, MT - mg * GS)
                    s_ps = spsum.tile([128, GS, NBLK], F32, tag="s")
                    for j in range(gs):
                        mt = mg * GS + j
                        MM(s_ps[:, j, :], k_sb[:, mt * 128:(mt + 1) * 128],
                           q_sb[:, nsl], start=True, stop=True)
                    et = etp.tile([128, GS, NBLK], BF16, tag="et")
                    nc.scalar.activation(et[:, :gs, :], s_ps[:, :gs, :], AF.Exp)
                    if prev is not None:
                        out_mms(*prev)
                    prev = (mg, et)
                out_mms(*prev)

                # evacuate PSUM promptly so the next n-block's accumulation
                # can start, then normalize off-PSUM:
                # outn = o / rowsum + gamma*bv
                o_sb = nrm.tile([128, CT, NBLK], F32, tag="osb")
                for ci in range(CT):
                    nc.scalar.copy(o_sb[:, ci, :], o_ps[ci])
                rs_sb = nrm.tile([1, NBLK], F32, tag="rssb")
                nc.scalar.copy(rs_sb, rs_ps)
                recip = nrm.tile([1, NBLK], F32, tag="recip")
                rscr = nrm.tile([1, NBLK], F32, tag="rscr")
                nc.vector.reciprocal_approx_accurate(recip, rs_sb, rscr)
                rb = nrm.tile([128, NBLK], F32, tag="rb")
                nc.gpsimd.partition_broadcast(rb, recip[0:1, :])
                for ci in range(CT):
                    tmp = nrm.tile([128, NBLK], F32, tag="tmp")
                    nc.vector.tensor_mul(tmp, o_sb[:, ci, :], rb)
                    nc.vector.tensor_scalar_add(outn[:, ci, nsl], tmp,
                                                gbv_sb[:, ci:ci + 1])

        # ---- phase D: bilinear 2x upsample + residual ----
        with tc.tile_pool(name="up", bufs=1) as up, \
             tc.tile_pool(name="ds", bufs=2) as ds:
            for ct in range(CT):
                A = outn[:, ct, :].rearrange("p (r q) -> p r q", q=w)
                q75 = up.tile([128, h, w], F32, tag="q75")
                nc.vector.tensor_scalar_mul(q75, A, 0.75)
                uw = up.tile([128, h, 2 * w], F32, tag="uw")
                stt = nc.vector.scalar_tensor_tensor
                # even output cols j=2t: 0.75*A[t] + 0.25*A[t-1] (clamped)
                stt(uw[:, :, 2::2], A[:, :, 0:w - 1], 0.25, q75[:, :, 1:w],
                    ALU.mult, ALU.add)
                stt(uw[:, :, 0:1], A[:, :, 0:1], 0.25, q75[:, :, 0:1],
                    ALU.mult, ALU.add)
                # odd output cols j=2t+1: 0.75*A[t] + 0.25*A[t+1] (clamped)
                stt(uw[:, :, 1:2 * w - 1:2], A[:, :, 1:w], 0.25,
                    q75[:, :, 0:w - 1], ALU.mult, ALU.add)
                stt(uw[:, :, 2 * w - 1:2 * w], A[:, :, w - 1:w], 0.25,
                    q75[:, :, w - 1:w], ALU.mult, ALU.add)

                for hs in range(SH2):
                    t0 = hs * 8
                    xs = ds.tile([128, 16, W], F32, tag="xs")
                    nc.sync.dma_start(
                        out=xs,
                        in_=x[ct * 128:(ct + 1) * 128,
                              hs * 16:(hs + 1) * 16, :])
                    r75 = ds.tile([128, 8, 2 * w], F32, tag="r75")
                    nc.vector.tensor_scalar_mul(r75, uw[:, t0:t0 + 8, :], 0.75)
                    fst = ds.tile([128, 16, 2 * w], F32, tag="fst")
                    # even out rows i=2t: 0.75*uw[t] + 0.25*uw[t-1]
                    if hs == 0:
                        stt(fst[:, 0:1, :], uw[:, 0:1, :], 0.25, r75[:, 0:1, :],
                            ALU.mult, ALU.add)
                        stt(fst[:, 2:16:2, :], uw[:, 0:7, :], 0.25,
                            r75[:, 1:8, :], ALU.mult, ALU.add)
                    else:
                        stt(fst[:, 0:16:2, :], uw[:, t0 - 1:t0 + 7, :], 0.25,
                            r75, ALU.mult, ALU.add)
                    # odd out rows i=2t+1: 0.75*uw[t] + 0.25*uw[t+1]
                    if hs < SH2 - 1:
                        stt(fst[:, 1:16:2, :], uw[:, t0 + 1:t0 + 9, :], 0.25,
                            r75, ALU.mult, ALU.add)
                    else:
                        stt(fst[:, 1:15:2, :], uw[:, t0 + 1:t0 + 8, :], 0.25,
                            r75[:, 0:7, :], ALU.mult, ALU.add)
                        stt(fst[:, 15:16, :], uw[:, t0 + 7:t0 + 8, :], 0.25,
                            r75[:, 7:8, :], ALU.mult, ALU.add)
                    fout = ds.tile([128, 16, W], F32, tag="fout")
                    nc.gpsimd.tensor_add(fout, fst, xs)
                    nc.sync.dma_start(
                        out=out[ct * 128:(ct + 1) * 128,
                                hs * 16:(hs + 1) * 16, :],
                        in_=fout)


_PROGRAMS = {}


def get_program(**kw):
    key = tuple(sorted(kw.items()))
    if key not in _PROGRAMS:
        _PROGRAMS[key] = build_program(**kw)
    return _PROGRAMS[key]


def make_in_maps(x, Wq, bq, Wk, bk, Wv, bv, gamma):
    g = float(np.asarray(gamma).reshape(-1)[0])
    bf = ml_dtypes.bfloat16
    wqt = np.ascontiguousarray((0.25 * Wq).T).astype(bf)
    wkt = np.ascontiguousarray((0.25 * Wk).T).astype(bf)
    wvt = np.ascontiguousarray((0.25 * g * Wv).T).astype(bf)
    bqv = np.asarray(bq, np.float32).reshape(-1, 1)
    gbv = (g * np.asarray(bv, np.float32)).reshape(1, -1)
    return [
        dict(x=np.ascontiguousarray(x[b]), wqt=wqt, wkt=wkt, wvt=wvt,
             bqv=bqv, gbv=gbv)
        for b in range(x.shape[0])
    ]


def kernel(x, Wq, bq, Wk, bk, Wv, bv, gamma):
    B, C, H, W = x.shape
    nc = get_program(C=C, H=H, W=W, D=Wq.shape[0], n_cores=B)
    in_maps = make_in_maps(x, Wq, bq, Wk, bk, Wv, bv, gamma)
    res = run_bass_kernel_spmd(nc, in_maps, core_ids=list(range(B)))
    return np.stack([res.results[b]["out"] for b in range(B)]).astype(np.float32)


# revision 7
# speedup vs baseline: 1.3444x; 1.3444x over previous
"""Trainium2 Bass kernel for nn_AttentionBlock (B=8, C=256, H=W=128).

Math (per batch element, data-parallel over 8 cores):
  xd = avgpool2x2(x)                      # [C, 64, 64] -> n=4096
  q = Wq xd + bq  (d=16);  k = Wk xd      # bk cancels in softmax (per-row const)
  S[n,m] = q_n . k_m ; attn = softmax_m(S) (no max-subtraction needed; |S| <~ 7)
  out = v @ attn^T where v = Wv xd + bv
  y = gamma*out upsampled bilinearly (half-pixel) + x

Device layout choices:
  - S is computed transposed (S^T[m,n]) so exp(S^T) tiles feed the output
    bmm directly as the moving operand with lhsT = v^T tiles.  The K=16
    score matmuls are packed 4x into the PE array via row tiling
    (tile_position), with q/k replicated into partition strips 0/32/64/96.
  - softmax denominator via ones-vector matmuls accumulated in PSUM.
  - bias bv and gamma folded: Wv,bv pre-scaled by gamma on host; bv added
    post-normalization as a per-partition scalar; the 1/4 avgpool scale is
    folded into Wq/Wk/Wv host-side.
  - bilinear 2x upsample is separable: per axis out = 0.75*a + 0.25*a_shift
    via scalar_tensor_tensor; residual add on gpsimd.  Upsample, residual
    and the output stores are streamed inside the attention n-block loop so
    they overlap PE work instead of forming a serial tail.
"""

import sys

sys.path.insert(0, "/opt/trn_rl_repo")

import numpy as np
import ml_dtypes

import concourse.bass as bass
import concourse.tile as tile
from concourse import bacc, mybir
from concourse.bass_utils import run_bass_kernel_spmd

AF = mybir.ActivationFunctionType
ALU = mybir.AluOpType
BF16 = mybir.dt.bfloat16
F32 = mybir.dt.float32


def build_program(C=256, H=128, W=128, D=16, n_cores=8):
    assert C == 256 and D == 16
    h, w = H // 2, W // 2
    n = h * w
    CT = C // 128          # c tiles (2)
    MT = n // 128          # m tiles
    NBLK = min(512, n)     # n block size
    NB = n // NBLK         # n blocks
    GS = min(4, MT)        # m-tiles per s-psum group (row-tiled pack of 4)
    SH = H // 16           # pooling strips (16 input rows each)
    SH2 = h // 8           # upsample strips (8 src rows -> 16 out rows)

    nc = bacc.Bacc("TRN2", target_bir_lowering=False, debug=False,
                   num_devices=n_cores)

    x = nc.dram_tensor("x", [C, H, W], F32, kind="ExternalInput").ap()
    wqt = nc.dram_tensor("wqt", [C, D], BF16, kind="ExternalInput").ap()
    wkt = nc.dram_tensor("wkt", [C, D], BF16, kind="ExternalInput").ap()
    wvt = nc.dram_tensor("wvt", [C, C], BF16, kind="ExternalInput").ap()
    bqv = nc.dram_tensor("bqv", [D, 1], F32, kind="ExternalInput").ap()
    gbv = nc.dram_tensor("gbv", [1, C], F32, kind="ExternalInput").ap()
    out = nc.dram_tensor("out", [C, H, W], F32, kind="ExternalOutput").ap()

    with tile.TileContext(nc) as tc:
        emit(tc, nc, x, wqt, wkt, wvt, bqv, gbv, out,
             C=C, H=H, W=W, D=D, h=h, w=w, n=n, CT=CT, MT=MT,
             NBLK=NBLK, NB=NB, GS=GS, SH=SH, SH2=SH2)

    nc.compile()
    return nc


def emit(tc, nc, x, wqt, wkt, wvt, bqv, gbv, out, *,
         C, H, W, D, h, w, n, CT, MT, NBLK, NB, GS, SH, SH2):
    MM = nc.tensor.matmul
    stt = nc.vector.scalar_tensor_tensor

    with tc.tile_pool(name="persist", bufs=1) as persist:
        # ---- persistent tiles ----
        xd = persist.tile([128, CT, n], BF16)          # pooled input, c-major
        q_rep = persist.tile([128, n], BF16)           # q at partitions 32j+0..15
        k_rep = persist.tile([128, n], BF16)
        vt_sb = persist.tile([128, MT, C], BF16)       # v^T tiles [m, c]
        wq_sb = persist.tile([128, CT, D], BF16)
        wk_sb = persist.tile([128, CT, D], BF16)
        wv_sb = persist.tile([128, CT, C], BF16)
        bq_sb = persist.tile([16, 1], F32)
        gbv_sb = persist.tile([128, CT], F32)          # gamma*bv per partition
        ones_sb = persist.tile([128, 1], BF16)
        # upsampled-in-w rows, per c tile: [h, 2w] (interleaved cols)
        uw = [persist.tile([128, h, 2 * w], BF16, name=f"uw{ci}")
              for ci in range(CT)]

        nc.sync.dma_start(out=wq_sb, in_=wqt.rearrange("(t p) d -> p t d", p=128))
        nc.sync.dma_start(out=wk_sb, in_=wkt.rearrange("(t p) d -> p t d", p=128))
        nc.sync.dma_start(out=wv_sb, in_=wvt.rearrange("(t p) c -> p t c", p=128))
        nc.sync.dma_start(out=bq_sb, in_=bqv)
        nc.sync.dma_start(out=gbv_sb, in_=gbv.rearrange("o (t p) -> p (t o)", p=128))
        nc.vector.memset(ones_sb, 1.0)

        # ---- phase A: load x, 2x2 average pool (scale folded into weights) ----
        with tc.tile_pool(name="xload", bufs=3) as xload:
            for ct in range(CT):
                for hs in range(SH):
                    xt = xload.tile([128, 16, W], F32, tag="xt")
                    nc.sync.dma_start(
                        out=xt,
                        in_=x[ct * 128:(ct + 1) * 128, hs * 16:(hs + 1) * 16, :])
                    t1 = xload.tile([128, 16, w], F32, tag="t1")
                    nc.gpsimd.tensor_add(t1, xt[:, :, 0::2], xt[:, :, 1::2])
                    xdv = xd[:, ct, hs * 8 * w:(hs + 1) * 8 * w].rearrange(
                        "p (r q) -> p r q", q=w)
                    nc.vector.tensor_add(xdv, t1[:, 0::2, :], t1[:, 1::2, :])

        # ---- phase B: projections q, k, v^T ----
        with tc.tile_pool(name="bpsum", bufs=2, space="PSUM") as bpsum:
            for nb in range(NB):
                sl = slice(nb * NBLK, (nb + 1) * NBLK)
                qp = bpsum.tile([16, NBLK], F32, tag="qk")
                MM(qp, wq_sb[:, 0, :], xd[:, 0, sl], start=True, stop=(CT == 1))
                if CT > 1:
                    MM(qp, wq_sb[:, 1, :], xd[:, 1, sl], start=False, stop=True)
                nc.scalar.activation(q_rep[0:16, sl], qp, AF.Identity, bias=bq_sb)
                kp = bpsum.tile([16, NBLK], F32, tag="qk")
                MM(kp, wk_sb[:, 0, :], xd[:, 0, sl], start=True, stop=(CT == 1))
                if CT > 1:
                    MM(kp, wk_sb[:, 1, :], xd[:, 1, sl], start=False, stop=True)
                nc.scalar.copy(k_rep[0:16, sl], kp)
            for mt in range(MT):
                msl = slice(mt * 128, (mt + 1) * 128)
                vp = bpsum.tile([128, C], F32, tag="v")
                MM(vp, xd[:, 0, msl], wv_sb[:, 0, :], start=True, stop=(CT == 1))
                if CT > 1:
                    MM(vp, xd[:, 1, msl], wv_sb[:, 1, :], start=False, stop=True)
                nc.scalar.copy(vt_sb[:, mt, :], vp)
        # replicate q/k into partition strips 32/64/96 for row-tiled matmuls
        for j in range(1, GS):
            nc.sync.dma_start(out=q_rep[32 * j:32 * j + 16, :], in_=q_rep[0:16, :])
            nc.sync.dma_start(out=k_rep[32 * j:32 * j + 16, :], in_=k_rep[0:16, :])

        # ---- phase C+D: attention, fused with upsample/residual streaming ----
        NG = (MT + GS - 1) // GS
        rows_per_nb = NBLK // w
        strips_done = 0

        def hpass(hs, last, ds):
            t0 = hs * 8
            for ci in range(CT):
                xs = ds.tile([128, 16, W], F32, tag="xs")
                nc.sync.dma_start(
                    out=xs,
                    in_=x[ci * 128:(ci + 1) * 128, hs * 16:(hs + 1) * 16, :])
                r75 = ds.tile([128, 8, 2 * w], BF16, tag="r75")
                nc.vector.tensor_scalar_mul(r75, uw[ci][:, t0:t0 + 8, :], 0.75)
                fst = ds.tile([128, 16, 2 * w], BF16, tag="fst")
                # even out rows i=2t: 0.75*uw[t] + 0.25*uw[t-1] (clamped)
                if hs == 0:
                    stt(fst[:, 0:1, :], uw[ci][:, 0:1, :], 0.25, r75[:, 0:1, :],
                        ALU.mult, ALU.add)
                    stt(fst[:, 2:16:2, :], uw[ci][:, 0:7, :], 0.25,
                        r75[:, 1:8, :], ALU.mult, ALU.add)
                else:
                    stt(fst[:, 0:16:2, :], uw[ci][:, t0 - 1:t0 + 7, :], 0.25,
                        r75, ALU.mult, ALU.add)
                # odd out rows i=2t+1: 0.75*uw[t] + 0.25*uw[t+1] (clamped)
                if not last:
                    stt(fst[:, 1:16:2, :], uw[ci][:, t0 + 1:t0 + 9, :], 0.25,
                        r75, ALU.mult, ALU.add)
                else:
                    stt(fst[:, 1:15:2, :], uw[ci][:, t0 + 1:t0 + 8, :], 0.25,
                        r75[:, 0:7, :], ALU.mult, ALU.add)
                    stt(fst[:, 15:16, :], uw[ci][:, t0 + 7:t0 + 8, :], 0.25,
                        r75[:, 7:8, :], ALU.mult, ALU.add)
                fout = ds.tile([128, 16, W], F32, tag="fout")
                nc.gpsimd.tensor_add(fout, fst, xs)
                nc.sync.dma_start(
                    out=out[ci * 128:(ci + 1) * 128, hs * 16:(hs + 1) * 16, :],
                    in_=fout)

        with tc.tile_pool(name="spsum", bufs=1, space="PSUM") as spsum, \
             tc.tile_pool(name="opsum", bufs=1, space="PSUM") as opsum, \
             tc.tile_pool(name="etp", bufs=2) as etp, \
             tc.tile_pool(name="nrm", bufs=2) as nrm, \
             tc.tile_pool(name="ds", bufs=2) as ds:
            for nb in range(NB):
                nsl = slice(nb * NBLK, (nb + 1) * NBLK)
                o_ps = [opsum.tile([128, NBLK], F32, tag=f"o{ci}",
                                   name=f"o_ps{ci}_{nb}")
                        for ci in range(CT)]
                rs_ps = opsum.tile([1, NBLK], F32, tag="rs")

                def out_mms(mg, et):
                    for j in range(GS):
                        mt = mg * GS + j
                        if mt >= MT:
                            break
                        first, last = (mt == 0), (mt == MT - 1)
                        for ci in range(CT):
                            MM(o_ps[ci], vt_sb[:, mt, ci * 128:(ci + 1) * 128],
                               et[:, j, :], start=first, stop=last)
                        MM(rs_ps, ones_sb, et[:, j, :], start=first, stop=last)

                prev = None
                for mg in range(NG):
                    gs = min(GS, MT - mg * GS)
                    s_ps = spsum.tile([128, GS, NBLK], F32, tag="s")
                    for j in range(gs):
                        mt = mg * GS + j
                        MM(s_ps[:, j, :],
                           k_rep[32 * j:32 * j + 16, mt * 128:(mt + 1) * 128],
                           q_rep[32 * j:32 * j + 16, nsl],
                           start=True, stop=True, tile_position=(32 * j, 0))
                    et = etp.tile([128, GS, NBLK], BF16, tag="et")
                    nc.scalar.activation(et[:, :gs, :], s_ps[:, :gs, :], AF.Exp)
                    if prev is not None:
                        out_mms(*prev)
                    prev = (mg, et)
                out_mms(*prev)

                # evacuate PSUM promptly, then normalize off-PSUM:
                # outn = o / rowsum + gamma*bv
                o_sb = nrm.tile([128, CT, NBLK], F32, tag="osb")
                for ci in range(CT):
                    nc.scalar.copy(o_sb[:, ci, :], o_ps[ci])
                rs_sb = nrm.tile([1, NBLK], F32, tag="rssb")
                nc.scalar.copy(rs_sb, rs_ps)
                recip = nrm.tile([1, NBLK], F32, tag="recip")
                rscr = nrm.tile([1, NBLK], F32, tag="rscr")
                nc.vector.reciprocal_approx_accurate(recip, rs_sb, rscr)
                rb = nrm.tile([128, NBLK], F32, tag="rb")
                nc.gpsimd.partition_broadcast(rb, recip[0:1, :])
                outn = nrm.tile([128, CT, NBLK], F32, tag="outn")
                for ci in range(CT):
                    nc.vector.tensor_mul(outn[:, ci, :], o_sb[:, ci, :], rb)
                    nc.vector.tensor_scalar_add(outn[:, ci, :], outn[:, ci, :],
                                                gbv_sb[:, ci:ci + 1])

                # w-direction upsample of the rows this n-block produced
                r0 = nb * rows_per_nb
                for ci in range(CT):
                    A = outn[:, ci, :].rearrange("p (r q) -> p r q", q=w)
                    q75 = nrm.tile([128, rows_per_nb, w], F32, tag="q75")
                    nc.vector.tensor_scalar_mul(q75, A, 0.75)
                    u = uw[ci][:, r0:r0 + rows_per_nb, :]
                    stt(u[:, :, 2::2], A[:, :, 0:w - 1], 0.25, q75[:, :, 1:w],
                        ALU.mult, ALU.add)
                    stt(u[:, :, 0:1], A[:, :, 0:1], 0.25, q75[:, :, 0:1],
                        ALU.mult, ALU.add)
                    stt(u[:, :, 1:2 * w - 1:2], A[:, :, 1:w], 0.25,
                        q75[:, :, 0:w - 1], ALU.mult, ALU.add)
                    stt(u[:, :, 2 * w - 1:2 * w], A[:, :, w - 1:w], 0.25,
                        q75[:, :, w - 1:w], ALU.mult, ALU.add)

                # h-direction upsample + residual + store for finished strips
                r1 = (nb + 1) * rows_per_nb
                while strips_done < SH2 - 1 and 8 * strips_done + 8 < r1:
                    hpass(strips_done, False, ds)
                    strips_done += 1
            hpass(SH2 - 1, True, ds)

_PROGRAMS = {}


def get_program(**kw):
    key = tuple(sorted(kw.items()))
    if key not in _PROGRAMS:
        _PROGRAMS[key] = build_program(**kw)
    return _PROGRAMS[key]


def make_in_maps(x, Wq, bq, Wk, bk, Wv, bv, gamma):
    g = float(np.asarray(gamma).reshape(-1)[0])
    bf = ml_dtypes.bfloat16
    wqt = np.ascontiguousarray((0.25 * Wq).T).astype(bf)
    wkt = np.ascontiguousarray((0.25 * Wk).T).astype(bf)
    wvt = np.ascontiguousarray((0.25 * g * Wv).T).astype(bf)
    bqv = np.asarray(bq, np.float32).reshape(-1, 1)
    gbv = (g * np.asarray(bv, np.float32)).reshape(1, -1)
    return [
        dict(x=np.ascontiguousarray(x[b]), wqt=wqt, wkt=wkt, wvt=wvt,
             bqv=bqv, gbv=gbv)
        for b in range(x.shape[0])
    ]


def kernel(x, Wq, bq, Wk, bk, Wv, bv, gamma):
    B, C, H, W = x.shape
    nc = get_program(C=C, H=H, W=W, D=Wq.shape[0], n_cores=B)
    in_maps = make_in_maps(x, Wq, bq, Wk, bk, Wv, bv, gamma)
    res = run_bass_kernel_spmd(nc, in_maps, core_ids=list(range(B)))
    return np.stack([res.results[b]["out"] for b in range(B)]).astype(np.float32)
